# revision 6
# baseline (speedup 1.0000x reference)
"""Trainium2 Bass kernel: batched bond-angle cosines (gather + vector math).

Problem: geometry (n_atoms, 3, batch) f32, angle triplets (n_angles, 3) int32.
Output: cos(angle) per (triplet, frame) = (n_angles, batch) f32.

Strategy: shard the angle dimension across 8 cores (8192 angles each). Each
core holds the full geometry in DRAM as a (n_atoms, 3*batch) row table and
uses indirect DMA (DGE gather) to pull the three endpoint-atom rows for each
128-angle tile into SBUF, then computes

    v1 = a - b ; v2 = c - b
    cos = dot(v1, v2) / sqrt(|v1|^2 * |v2|^2)

with vector/scalar/gpsimd engines. Output rows are contiguous per core.
"""

import numpy as np

import concourse.tile as tile
from concourse import bacc, bass, mybir
from concourse.bass_utils import run_bass_kernel_spmd

P = 128

N_ATOMS = 2048
N_ANGLES = 65536
BATCH = 512
N_CORES = 8
PER_CORE = N_ANGLES // N_CORES  # 8192
N_TILES = PER_CORE // P  # 64

_NC_CACHE = {}


def build_nc(n_atoms=N_ATOMS, per_core=PER_CORE, batch=BATCH):
    """Build the single-core SPMD Bass program (same program on all cores;
    per-core behavior differs only through the 'idxs' input values)."""
    n_tiles = per_core // P
    B = batch
    f32 = mybir.dt.float32
    i32 = mybir.dt.int32

    nc = bacc.Bacc(debug=False)

    geom = nc.declare_dram_parameter("geom", [n_atoms, 3 * B], f32, isOutput=False)
    # idxs[p, t*3 + r] = angles[t*128 + p, r]  (r: 0=left,1=center,2=right)
    idxs = nc.declare_dram_parameter("idxs", [P, 3 * n_tiles], i32, isOutput=False)
    out = nc.declare_dram_parameter("out", [per_core, B], f32, isOutput=True)

    with tile.TileContext(nc) as tc:
        with (
            tc.tile_pool(name="idxp", bufs=1) as idxp,
            tc.tile_pool(name="gath", bufs=3) as gath,
            tc.tile_pool(name="work", bufs=2) as work,
            tc.tile_pool(name="outp", bufs=3) as outp,
        ):
            idx_sb = idxp.tile([P, 3 * n_tiles], i32)
            nc.sync.dma_start(out=idx_sb[:, :], in_=idxs[:, :])

            for t in range(n_tiles):
                ga = gath.tile([P, 3 * B], f32, tag="ga")
                gb = gath.tile([P, 3 * B], f32, tag="gb")
                gc = gath.tile([P, 3 * B], f32, tag="gc")
                for role, g in enumerate((ga, gb, gc)):
                    nc.gpsimd.indirect_dma_start(
                        out=g[:, :],
                        out_offset=None,
                        in_=geom[:, :],
                        in_offset=bass.IndirectOffsetOnAxis(
                            ap=idx_sb[:, 3 * t + role : 3 * t + role + 1],
                            axis=0,
                        ),
                    )

                # coordinate slices
                ax, ay, az = (ga[:, i * B : (i + 1) * B] for i in range(3))
                bx, by, bz = (gb[:, i * B : (i + 1) * B] for i in range(3))
                cx, cy, cz = (gc[:, i * B : (i + 1) * B] for i in range(3))

                d1 = work.tile([P, 3 * B], f32, tag="d1")
                d2 = work.tile([P, 3 * B], f32, tag="d2")
                dx1, dy1, dz1 = (d1[:, i * B : (i + 1) * B] for i in range(3))
                dx2, dy2, dz2 = (d2[:, i * B : (i + 1) * B] for i in range(3))

                # v1 = a - b, v2 = c - b  (6 DVE subs)
                nc.vector.tensor_sub(dx1, ax, bx)
                nc.vector.tensor_sub(dy1, ay, by)
                nc.vector.tensor_sub(dz1, az, bz)
                nc.vector.tensor_sub(dx2, cx, bx)
                nc.vector.tensor_sub(dy2, cy, by)
                nc.vector.tensor_sub(dz2, cz, bz)

                dot = work.tile([P, B], f32, tag="dot")
                tmp = work.tile([P, B], f32, tag="tmp")
                n1 = work.tile([P, B], f32, tag="n1")
                n2 = work.tile([P, B], f32, tag="n2")
                sq = work.tile([P, B], f32, tag="sq")

                # dot = dx1*dx2 + dy1*dy2 + dz1*dz2
                nc.vector.tensor_mul(dot, dx1, dx2)
                nc.vector.tensor_mul(tmp, dy1, dy2)
                nc.vector.tensor_add(dot, dot, tmp)
                nc.vector.tensor_mul(tmp, dz1, dz2)
                nc.vector.tensor_add(dot, dot, tmp)

                # |v1|^2 (squares on ACT, adds on gpsimd)
                nc.scalar.square(n1, dx1)
                nc.scalar.square(sq, dy1)
                nc.gpsimd.tensor_add(n1, n1, sq)
                nc.scalar.square(sq, dz1)
                nc.gpsimd.tensor_add(n1, n1, sq)

                # |v2|^2
                nc.scalar.square(n2, dx2)
                nc.scalar.square(sq, dy2)
                nc.gpsimd.tensor_add(n2, n2, sq)
                nc.scalar.square(sq, dz2)
                nc.gpsimd.tensor_add(n2, n2, sq)

                # denom = sqrt(n1*n2); res = dot * (1/denom)
                nc.vector.tensor_mul(n1, n1, n2)
                nc.scalar.sqrt(n1, n1)
                nc.vector.reciprocal(n2, n1)

                res = outp.tile([P, B], f32, tag="res")
                nc.vector.tensor_mul(res[:, :], dot[:, :], n2[:, :])

                nc.sync.dma_start(
                    out=out[t * P : (t + 1) * P, :], in_=res[:, :]
                )

    nc.compile()
    return nc


def _prep_core_inputs(geom2d, angles, core):
    ang = angles[core * PER_CORE : (core + 1) * PER_CORE]
    idxs = np.ascontiguousarray(
        ang.reshape(N_TILES, P, 3).transpose(1, 0, 2).reshape(P, 3 * N_TILES)
    )
    return {"geom": geom2d, "idxs": idxs}


def kernel(input, angles, _trace=False, _trace_kwargs=None):
    input = np.ascontiguousarray(np.asarray(input, dtype=np.float32))
    angles = np.ascontiguousarray(np.asarray(angles, dtype=np.int32))
    assert input.shape == (N_ATOMS, 3, BATCH)
    assert angles.shape == (N_ANGLES, 3)

    geom2d = input.reshape(N_ATOMS, 3 * BATCH)

    key = (N_ATOMS, PER_CORE, BATCH)
    if key not in _NC_CACHE:
        _NC_CACHE[key] = build_nc(*key)
    nc = _NC_CACHE[key]

    in_maps = [_prep_core_inputs(geom2d, angles, c) for c in range(N_CORES)]
    kw = {}
    if _trace:
        kw["trace"] = True
        kw.update(_trace_kwargs or {})
    res = run_bass_kernel_spmd(nc, in_maps, core_ids=list(range(N_CORES)), **kw)
    outs = [res.results[c]["out"] for c in range(N_CORES)]
    full = np.concatenate(outs, axis=0)
    if _trace:
        return full, res
    return full
